# revision 2
# baseline (speedup 1.0000x reference)
"""PointNet++ feature propagation on 8 NeuronCores (batch-parallel), v2.

Per core (one batch):
- CPU pre-sorts queries and prev points by z; each 128-query tile only scans a
  fixed 768-wide window of z-sorted prev points (covers the 3-NN ball with
  margin; the few expected misses are far below the error budget).
- Exact-f32-class distances via split-fp32r matmul (16-row trick) -> PSUM.
- top-8 scan (DVE max8/max_index8 on PSUM window), top-3 -> inverse-distance
  weights.
- Interpolation: indirect-DMA gather of W0-projected features_prev (fp16 Z
  table, 24 rows per query-group per instruction), weighted via diagonal
  matmuls accumulated into conv0's PSUM. Diagonals built on GPSIMD.
- conv0/conv1 in fp16 (fp32 PSUM accum), training-mode BN with cross-core
  AllReduce of stats, fused scale+bias+ReLU on ACT.
- Software pipeline: conv work for group g-1 is emitted after scan+gather of
  group g so the DVE scan (critical path) never starves.
"""
import os
import numpy as np

import concourse.bass as bass
import concourse.mybir as mybir
import concourse.tile as tile_mod
from concourse import tile
from concourse.bass_utils import run_bass_kernel_spmd
from concourse.vector_clock import ScopedClock
from concourse.bass import _add_dep_helper

dt = mybir.dt

B, N, M, C, CP = 8, 8192, 2048, 256, 512
O0, O1 = 256, 128
NT = N // 128           # 64 query tiles
W = 768                 # z-window of prev candidates per tile
GRP = 8                 # tiles per scan group
NG = NT // GRP          # 8 groups
CB = 4                  # tiles per conv/psum batch
NB = NT // CB           # 16 conv batches
NCH = 8                 # phase2/3 chunks of 1024 queries
BN_EPS = 1e-5

C0S = [min(max(32 * t + 16 - W // 2, 0), M - W) for t in range(NT)]

# ---------------------------------------------------------------------------
# workarounds: this walrus build accepts at most ONE sync wait per instruction
MAX_WAITS = 1
_wsplit_ctr = [0]


def _patched_drain_and_barrier(self, tick_clock, wait_clock):
    nc = self.nc
    drain_inst = nc.sync.drain()
    wait_clock.add_sem_waits(
        drain_inst.ins, ScopedClock({None: tick_clock.global_clock})
    )
    si = drain_inst.ins.sync_info
    if si is not None and si.on_wait is not None and len(si.on_wait) > MAX_WAITS:
        waits = list(si.on_wait)
        si.on_wait = waits[:MAX_WAITS]
        for _ in range(MAX_WAITS, len(waits), MAX_WAITS):
            extra = nc.sync.drain()
            if extra.ins.sync_info is None:
                extra.ins.sync_info = mybir.SyncInfo(on_wait=[], on_update=[])
        bb = nc.cur_bb.bb if hasattr(nc.cur_bb, "bb") else nc.cur_bb
        seen = False
        idx = MAX_WAITS
        for inst in bb.instructions:
            if inst is drain_inst.ins or inst.name == drain_inst.ins.name:
                seen = True
                continue
            if seen and inst.opcode == "Drain" and idx < len(waits):
                inst.sync_info.on_wait = waits[idx:idx + MAX_WAITS]
                idx += MAX_WAITS
    nc.all_engine_barrier()
    popped = nc._tile_sem_poison_stack.pop()
    assert popped is self._sem_poison
    nc.clear_and_free_semaphores(list(self.sems.allocated().values()))
    nc.all_engine_barrier()


tile_mod.TileContext._drain_and_barrier = _patched_drain_and_barrier


def _split_multi_waits(nc):
    for f in nc.m.functions:
        for bb in f.blocks:
            new_insts = []
            changed = False
            for inst in bb.instructions:
                si = inst.sync_info
                waits = list(si.on_wait) if (si is not None and si.on_wait) else []
                if len(waits) > MAX_WAITS and (
                        inst.engine != mybir.EngineType.SP
                        or inst.opcode == "DMACopy"):
                    changed = True
                    extra, keep = waits[:-MAX_WAITS], waits[-MAX_WAITS:]
                    for j in range(0, len(extra), MAX_WAITS):
                        _wsplit_ctr[0] += 1
                        nop = mybir.InstNoOp(
                            name=f"WSPLIT-{_wsplit_ctr[0]}", ins=[], outs=[])
                        nop.engine = inst.engine
                        nop.sync_info = mybir.SyncInfo(
                            on_wait=extra[j:j + MAX_WAITS], on_update=[])
                        new_insts.append(nop)
                    si.on_wait = keep
                new_insts.append(inst)
            if changed:
                bb.instructions.clear()
                for i in new_insts:
                    bb.add_instruction(i)


# ---------------------------------------------------------------------------
def _build_nc():
    nc = bass.Bass("TRN2", target_bir_lowering=False, debug=False, num_devices=8)

    d_lhs = nc.dram_tensor("lhs16", [16, N], dt.float32r, kind="ExternalInput")
    d_rhs = nc.dram_tensor("rhs16", [16, M], dt.float32r, kind="ExternalInput")
    d_feat = nc.dram_tensor("feat", [C, N], dt.float16, kind="ExternalInput")
    d_fp = nc.dram_tensor("fp", [CP, M], dt.float16, kind="ExternalInput")
    d_w0pt = nc.dram_tensor("w0pt", [CP, O0], dt.float16, kind="ExternalInput")
    d_w0ft = nc.dram_tensor("w0ft", [C, O0], dt.float16, kind="ExternalInput")
    d_w1t = nc.dram_tensor("w1t", [O0, O1], dt.float16, kind="ExternalInput")
    d_eye3 = nc.dram_tensor("eye3", [128, 3 * 128], dt.float32, kind="ExternalInput")
    d_c0b = nc.dram_tensor("c0b", [128, NT], dt.uint32, kind="ExternalInput")
    d_g0 = nc.dram_tensor("g0", [128, 2], dt.float32, kind="ExternalInput")
    d_be0 = nc.dram_tensor("be0", [128, 2], dt.float32, kind="ExternalInput")
    d_g1 = nc.dram_tensor("g1", [128, 1], dt.float32, kind="ExternalInput")
    d_be1 = nc.dram_tensor("be1", [128, 1], dt.float32, kind="ExternalInput")
    d_out = nc.dram_tensor("out", [O1, N], dt.float32, kind="ExternalOutput")

    KDEBUG = os.environ.get("KDEBUG", "0") == "1"
    if KDEBUG:
        d_dbg_v = nc.dram_tensor("dbg_vals", [128, NT * 8], dt.float32, kind="ExternalOutput")
        d_dbg_i = nc.dram_tensor("dbg_idxs", [128, NT * 8], dt.uint32, kind="ExternalOutput")
        d_dbg_ig = nc.dram_tensor("dbg_idxg", [128, NT * 3], dt.uint32, kind="ExternalOutput")
        d_dbg_w = nc.dram_tensor("dbg_w", [128, NT * 3], dt.float32, kind="ExternalOutput")
        d_dbg_y0 = nc.dram_tensor("dbg_y0", [2, 128, N], dt.float16, kind="ExternalOutput")
        d_dbg_zt = nc.dram_tensor("dbg_zt", [M, O0], dt.float16, kind="ExternalOutput")
        d_dbg_gt = nc.dram_tensor("dbg_gt", [128, GRP * 3 * O0], dt.float16, kind="ExternalOutput")
        d_dbg_dj = nc.dram_tensor("dbg_dj", [128, 3 * 128], dt.float16, kind="ExternalOutput")

    d_zt = nc.dram_tensor("ztab", [M, O0], dt.float16)
    d_ar0i = nc.dram_tensor("ar0i", [128, 4], dt.float32)
    d_ar0o = nc.dram_tensor("ar0o", [128, 4], dt.float32, addr_space="Shared")
    d_ar1i = nc.dram_tensor("ar1i", [128, 2], dt.float32)
    d_ar1o = nc.dram_tensor("ar1o", [128, 2], dt.float32, addr_space="Shared")

    RG = [[0, 1, 2, 3, 4, 5, 6, 7]]
    AF = mybir.ActivationFunctionType
    ALU = mybir.AluOpType

    with tile.TileContext(nc) as tc:
        with tc.tile_pool(name="persist", bufs=1) as pp, \
             tc.tile_pool(name="spool", bufs=2, space="PSUM") as sp, \
             tc.tile_pool(name="ypool", bufs=2, space="PSUM") as yp:

            # ---------- loads (SP-issued HWDGE, few big DMAs) ----------
            lhs_r = pp.tile([16, N], dt.float32r)
            nc.sync.dma_start(lhs_r[:], d_lhs[:])
            rhs_r = pp.tile([16, M], dt.float32r)
            nc.sync.dma_start(rhs_r[:], d_rhs[:])
            featsb = []
            for kc in range(2):
                t_f = pp.tile([128, N], dt.float16, tag=f"feat{kc}", name=f"feat{kc}")
                nc.sync.dma_start(t_f[:], d_feat[128 * kc:128 * (kc + 1), :])
                featsb.append(t_f)
            w0f_sb = []
            for kc in range(2):
                t_w = pp.tile([128, O0], dt.float16, tag=f"w0f{kc}", name=f"w0f{kc}")
                nc.sync.dma_start(t_w[:], d_w0ft[128 * kc:128 * (kc + 1), :])
                w0f_sb.append(t_w)
            w1t_sb = []
            for kc in range(2):
                t_w = pp.tile([128, O1], dt.float16, tag=f"w1t{kc}", name=f"w1t{kc}")
                nc.sync.dma_start(t_w[:], d_w1t[128 * kc:128 * (kc + 1), :])
                w1t_sb.append(t_w)
            t_eye3 = pp.tile([128, 3, 128], dt.float32)
            nc.sync.dma_start(
                t_eye3[:].rearrange("p a b -> p (a b)"), d_eye3[:])
            t_c0b = pp.tile([128, NT], dt.uint32)
            nc.sync.dma_start(t_c0b[:], d_c0b[:])
            t_g0 = pp.tile([128, 2], dt.float32)
            nc.sync.dma_start(t_g0[:], d_g0[:])
            t_be0 = pp.tile([128, 2], dt.float32)
            nc.sync.dma_start(t_be0[:], d_be0[:])
            t_g1 = pp.tile([128, 1], dt.float32)
            nc.sync.dma_start(t_g1[:], d_g1[:])
            t_be1 = pp.tile([128, 1], dt.float32)
            nc.sync.dma_start(t_be1[:], d_be1[:])

            # ---------- Z table: ZT[m, o] = sum_c fp[c, m] * W0pT[c, o] ----------
            zsb = pp.tile([128, 16, O0], dt.float16, name="zsb")
            with tc.tile_pool(name="zbuild", bufs=1) as zb:
                w0p_sb = []
                for kc in range(4):
                    t_w = zb.tile([128, O0], dt.float16, tag=f"w0p{kc}")
                    nc.sync.dma_start(t_w[:], d_w0pt[128 * kc:128 * (kc + 1), :])
                    w0p_sb.append(t_w)
                fp_sb = []
                for kc in range(4):
                    t_f = zb.tile([128, M], dt.float16, tag=f"fp{kc}")
                    nc.sync.dma_start(t_f[:], d_fp[128 * kc:128 * (kc + 1), :])
                    fp_sb.append(t_f)
                for mb in range(4):          # 4 m-tile batches of 4
                    zps = yp.tile([128, CB, O0], dt.float32, tag="yps")
                    for mi in range(4):
                        mt = 4 * mb + mi
                        for kc in range(4):
                            nc.tensor.matmul(
                                zps[:, mi, :],
                                fp_sb[kc][:, 128 * mt:128 * (mt + 1)],
                                w0p_sb[kc][:],
                                start=(kc == 0), stop=(kc == 3))
                    nc.scalar.activation(
                        zsb[:, 4 * mb:4 * (mb + 1), :].rearrange("p a b -> p (a b)"),
                        zps[:].rearrange("p a b -> p (a b)"), AF.Copy)
            zt_store = nc.sync.dma_start(
                d_zt[:].rearrange("(a p) o -> p a o", p=128), zsb[:])
            zt_token = pp.tile([1, 1], dt.float32)
            tok = nc.gpsimd.memset(zt_token[:], 0.0)
            _add_dep_helper(tok.ins, zt_store.ins, sync=True, reason="zt ready")

            # ---------- persistent state ----------
            vals = pp.tile([128, NT * 8], dt.float32)
            idxs = pp.tile([128, NT, 8], dt.uint32)
            idxg = pp.tile([128, NT, 3], dt.uint32)
            wgall = pp.tile([128, NT, 3], dt.float32)
            y0raw = []
            for oc in range(2):
                y0r = pp.tile([128, N], dt.float16, tag=f"y0raw{oc}", name=f"y0raw{oc}")
                y0raw.append(y0r)
            y1raw = pp.tile([128, N], dt.float16, name="y1raw")
            s0sl = pp.tile([128, 2 * NB], dt.float32)
            q0sl = pp.tile([128, 2 * NB], dt.float32)
            s1sl = pp.tile([128, NCH], dt.float32)
            q1sl = pp.tile([128, NCH], dt.float32)

            # ---------- phase 1: software-pipelined over groups ----------
            with tc.tile_pool(name="p1", bufs=2) as p1, \
                 tc.tile_pool(name="pdj", bufs=2 * GRP) as pdj, \
                 tc.tile_pool(name="pscrap", bufs=2) as pscrap:
                gts = {}
                for g in range(NG + 1):
                    if g < NG:
                        # --- distance + top-8 scan for group g ---
                        for r in range(GRP):
                            t = GRP * g + r
                            c0 = C0S[t]
                            qsl = slice(128 * t, 128 * (t + 1))
                            sps = sp.tile([128, W], dt.float32, tag="sps")
                            nc.tensor.matmul(
                                sps[:, 0:512], lhs_r[:, qsl],
                                rhs_r[:, c0:c0 + 512], start=True, stop=True)
                            nc.tensor.matmul(
                                sps[:, 512:W], lhs_r[:, qsl],
                                rhs_r[:, c0 + 512:c0 + W], start=True, stop=True)
                            vsl = vals[:, 8 * t:8 * (t + 1)]
                            nc.vector.max(vsl, sps[:])
                            nc.vector.max_index(
                                idxs[:, t, :], vsl, sps[:])

                        gsl = slice(GRP * g, GRP * (g + 1))
                        v_view = vals[:, 64 * g:64 * (g + 1)].rearrange(
                            "p (t e) -> p t e", e=8)[:, :, 0:3]
                        # d2 = max(-s, 1e-12)
                        d2 = p1.tile([128, GRP, 3], dt.float32, tag="d2")
                        nc.vector.tensor_scalar(
                            d2[:], v_view, -1.0, 1e-12,
                            op0=ALU.mult, op1=ALU.max)
                        inv = p1.tile([128, GRP, 3], dt.float32, tag="inv")
                        nc.vector.reciprocal(inv[:], d2[:])
                        ws = p1.tile([128, GRP], dt.float32, tag="ws")
                        nc.vector.tensor_reduce(
                            ws[:], inv[:], op=ALU.add, axis=mybir.AxisListType.X)
                        wsi = p1.tile([128, GRP], dt.float32, tag="wsi")
                        nc.vector.reciprocal(wsi[:], ws[:])
                        nc.vector.tensor_tensor(
                            wgall[:, gsl, :], inv[:],
                            wsi[:].rearrange("p (t o) -> p t o", o=1)
                                  .broadcast_to([128, GRP, 3]),
                            op=ALU.mult)
                        # global indices = local + window start
                        nc.vector.tensor_tensor(
                            idxg[:, gsl, :], idxs[:, gsl, 0:3],
                            t_c0b[:, gsl].rearrange("p (t o) -> p t o", o=1)
                                 .broadcast_to([128, GRP, 3]),
                            op=ALU.add)
                        # diag weight matrices for group g (GPSIMD)
                        for r in range(GRP):
                            t = GRP * g + r
                            dj = pdj.tile([128, 3, 128], dt.float16, tag="dj",
                                          name=f"dj{t}")
                            nc.gpsimd.tensor_tensor(
                                dj[:], t_eye3[:],
                                wgall[:, t, :].rearrange("p (e o) -> p e o", o=1)
                                     .broadcast_to([128, 3, 128]),
                                op=ALU.mult)
                            gts.setdefault(g, []).append(dj)
                        # gather Z rows for group g (single-offset indirect
                        # DMAs -- the only variant this walrus build executes
                        # correctly)
                        gt = p1.tile([128, GRP, 3, O0], dt.float16, tag="gt",
                                     name=f"gt{g}")
                        for r in range(GRP):
                            t = GRP * g + r
                            for j in range(3):
                                gi = nc.gpsimd.indirect_dma_start(
                                    out=gt[:, r, j, :], out_offset=None,
                                    in_=d_zt[:],
                                    in_offset=bass.IndirectOffsetOnAxis(
                                        ap=idxg[:, t, j:j + 1], axis=0))
                                _add_dep_helper(gi.ins, tok.ins, sync=True,
                                                reason="gather after zt")
                        if KDEBUG and g == 0:
                            nc.sync.dma_start(
                                d_dbg_gt[:],
                                gt[:].rearrange("p a b o -> p (a b o)"))
                            nc.sync.dma_start(
                                d_dbg_dj[:],
                                gts[g][0][:].rearrange("p a b -> p (a b)"))
                        gts[g] = (gt, gts[g])

                    if g >= 1:
                        gg = g - 1
                        gt, djs = gts.pop(gg)
                        for bb_ in range(2):
                            bi = 2 * gg + bb_
                            q0 = 128 * (GRP * gg + CB * bb_)
                            yps = yp.tile([128, 2, CB, 128], dt.float32, tag="yps")
                            for oc in range(2):
                                osl = slice(128 * oc, 128 * (oc + 1))
                                for kc in range(2):
                                    nc.tensor.matmul(
                                        yps[:, oc, :, :].rearrange("p a b -> p (a b)"),
                                        w0f_sb[kc][:, osl],
                                        featsb[kc][:, q0:q0 + 512 - 0],
                                        start=(kc == 0), stop=False)
                            for r in range(CB):
                                ti = CB * bb_ + r
                                dj = djs[ti]
                                for oc in range(2):
                                    osl = slice(128 * oc, 128 * (oc + 1))
                                    for j in range(3):
                                        nc.tensor.matmul(
                                            yps[:, oc, r, :],
                                            gt[:, ti, j, osl], dj[:, j, :],
                                            start=False, stop=(j == 2))
                            # copy + stats (ACT)
                            for oc in range(2):
                                nc.scalar.activation(
                                    y0raw[oc][:, q0:q0 + 512],
                                    yps[:, oc, :, :].rearrange("p a b -> p (a b)"),
                                    AF.Copy,
                                    accum_out=s0sl[:, NB * oc + bi:NB * oc + bi + 1])
                                scrap = pscrap.tile([128, 512], dt.float16,
                                                    tag="scrap")
                                nc.scalar.activation(
                                    scrap[:],
                                    yps[:, oc, :, :].rearrange("p a b -> p (a b)"),
                                    AF.Square,
                                    accum_out=q0sl[:, NB * oc + bi:NB * oc + bi + 1])

                # ---------- allreduce layer-0 stats ----------
                ar0 = pp.tile([128, 4], dt.float32)
                nc.vector.tensor_reduce(
                    ar0[:, 0:1], s0sl[:, 0:NB], op=ALU.add,
                    axis=mybir.AxisListType.X)
                nc.vector.tensor_reduce(
                    ar0[:, 1:2], s0sl[:, NB:2 * NB], op=ALU.add,
                    axis=mybir.AxisListType.X)
                nc.vector.tensor_reduce(
                    ar0[:, 2:3], q0sl[:, 0:NB], op=ALU.add,
                    axis=mybir.AxisListType.X)
                nc.vector.tensor_reduce(
                    ar0[:, 3:4], q0sl[:, NB:2 * NB], op=ALU.add,
                    axis=mybir.AxisListType.X)
                st0 = nc.gpsimd.dma_start(d_ar0i[:], ar0[:])
                cc0 = nc.gpsimd.collective_compute(
                    "AllReduce", ALU.add, replica_groups=RG,
                    ins=[d_ar0i[:]], outs=[d_ar0o[:]])
                _add_dep_helper(cc0.ins, st0.ins, sync=True, reason="ar0 in")
                ar0r = pp.tile([128, 4], dt.float32)
                ld0 = nc.gpsimd.dma_start(ar0r[:], d_ar0o[:])
                _add_dep_helper(ld0.ins, cc0.ins, sync=True, reason="ar0 out")

                cnt = float(B * N)
                mean0 = pp.tile([128, 2], dt.float32)
                nc.vector.tensor_scalar_mul(mean0[:], ar0r[:, 0:2], 1.0 / cnt)
                var0 = pp.tile([128, 2], dt.float32)
                nc.vector.tensor_scalar_mul(var0[:], ar0r[:, 2:4], 1.0 / cnt)
                msq0 = pp.tile([128, 2], dt.float32)
                nc.vector.tensor_tensor(
                    msq0[:], mean0[:], mean0[:], op=ALU.mult)
                nc.vector.tensor_tensor(
                    var0[:], var0[:], msq0[:], op=ALU.subtract)
                nc.vector.tensor_scalar_add(var0[:], var0[:], BN_EPS)
                sd0 = pp.tile([128, 2], dt.float32)
                nc.scalar.activation(sd0[:], var0[:], AF.Sqrt)
                isd0 = pp.tile([128, 2], dt.float32)
                nc.vector.reciprocal(isd0[:], sd0[:])
                a0 = pp.tile([128, 2], dt.float32)
                nc.vector.tensor_tensor(a0[:], t_g0[:], isd0[:], op=ALU.mult)
                c0t = pp.tile([128, 2], dt.float32)
                nc.vector.tensor_tensor(c0t[:], mean0[:], a0[:], op=ALU.mult)
                nc.vector.tensor_tensor(c0t[:], t_be0[:], c0t[:], op=ALU.subtract)

            if KDEBUG:
                nc.sync.dma_start(d_dbg_v[:], vals[:])
                nc.sync.dma_start(d_dbg_i[:], idxs[:].rearrange("p a b -> p (a b)"))
                nc.sync.dma_start(d_dbg_ig[:], idxg[:].rearrange("p a b -> p (a b)"))
                nc.sync.dma_start(d_dbg_w[:], wgall[:].rearrange("p a b -> p (a b)"))
                for oc in range(2):
                    nc.sync.dma_start(d_dbg_y0[oc], y0raw[oc][:])
                nc.sync.dma_start(d_dbg_zt[:].rearrange("(a p) o -> p a o", p=128), zsb[:])

            # ---------- phase 2: h = relu(a0*y0+c0); y1 = W1 @ h ----------
            with tc.tile_pool(name="p2", bufs=2) as p2:
                for ci in range(NCH):
                    qsl = slice(1024 * ci, 1024 * (ci + 1))
                    h = []
                    for oc in range(2):
                        h_t = p2.tile([128, 1024], dt.float16, tag=f"h{oc}",
                                      name=f"h{oc}_{ci}")
                        nc.scalar.activation(
                            h_t[:], y0raw[oc][:, qsl], AF.Relu,
                            scale=a0[:, oc:oc + 1], bias=c0t[:, oc:oc + 1])
                        h.append(h_t)
                    y1ps = yp.tile([128, 2, CB, 128], dt.float32, tag="yps")
                    y1flat = y1ps[:].rearrange("p a b c -> p (a b c)")
                    for hf in range(2):
                        hsl = slice(512 * hf, 512 * (hf + 1))
                        for kc in range(2):
                            nc.tensor.matmul(
                                y1flat[:, hsl], w1t_sb[kc][:], h[kc][:, hsl],
                                start=(kc == 0), stop=(kc == 1))
                    nc.scalar.activation(
                        y1raw[:, qsl], y1flat, AF.Copy,
                        accum_out=s1sl[:, ci:ci + 1])
                    sq = p2.tile([128, 1024], dt.float16, tag="sq")
                    nc.vector.scalar_tensor_tensor(
                        sq[:], y1raw[:, qsl], 1.0, y1raw[:, qsl],
                        op0=ALU.mult, op1=ALU.mult,
                        accum_out=q1sl[:, ci:ci + 1])

                ar1 = pp.tile([128, 2], dt.float32)
                nc.vector.tensor_reduce(
                    ar1[:, 0:1], s1sl[:], op=ALU.add, axis=mybir.AxisListType.X)
                nc.vector.tensor_reduce(
                    ar1[:, 1:2], q1sl[:], op=ALU.add, axis=mybir.AxisListType.X)
                st1 = nc.gpsimd.dma_start(d_ar1i[:], ar1[:])
                cc1 = nc.gpsimd.collective_compute(
                    "AllReduce", ALU.add, replica_groups=RG,
                    ins=[d_ar1i[:]], outs=[d_ar1o[:]])
                _add_dep_helper(cc1.ins, st1.ins, sync=True, reason="ar1 in")
                ar1r = pp.tile([128, 2], dt.float32)
                ld1 = nc.gpsimd.dma_start(ar1r[:], d_ar1o[:])
                _add_dep_helper(ld1.ins, cc1.ins, sync=True, reason="ar1 out")

                mean1 = pp.tile([128, 1], dt.float32)
                nc.vector.tensor_scalar_mul(mean1[:], ar1r[:, 0:1], 1.0 / cnt)
                var1 = pp.tile([128, 1], dt.float32)
                nc.vector.tensor_scalar_mul(var1[:], ar1r[:, 1:2], 1.0 / cnt)
                msq1 = pp.tile([128, 1], dt.float32)
                nc.vector.tensor_tensor(msq1[:], mean1[:], mean1[:], op=ALU.mult)
                nc.vector.tensor_tensor(var1[:], var1[:], msq1[:], op=ALU.subtract)
                nc.vector.tensor_scalar_add(var1[:], var1[:], BN_EPS)
                sd1 = pp.tile([128, 1], dt.float32)
                nc.scalar.activation(sd1[:], var1[:], AF.Sqrt)
                isd1 = pp.tile([128, 1], dt.float32)
                nc.vector.reciprocal(isd1[:], sd1[:])
                a1 = pp.tile([128, 1], dt.float32)
                nc.vector.tensor_tensor(a1[:], t_g1[:], isd1[:], op=ALU.mult)
                c1 = pp.tile([128, 1], dt.float32)
                nc.vector.tensor_tensor(c1[:], mean1[:], a1[:], op=ALU.mult)
                nc.vector.tensor_tensor(c1[:], t_be1[:], c1[:], op=ALU.subtract)

            # ---------- phase 3: out = relu(a1*y1+c1) ----------
            with tc.tile_pool(name="p3", bufs=2) as p3:
                for ci in range(NCH):
                    qsl = slice(1024 * ci, 1024 * (ci + 1))
                    o = p3.tile([128, 1024], dt.float32, tag="o")
                    nc.scalar.activation(o[:], y1raw[:, qsl], AF.Relu,
                                         scale=a1[:], bias=c1[:])
                    nc.sync.dma_start(d_out[:, qsl], o[:])

    _split_multi_waits(nc)
    return nc


_NC_CACHE = []


def _get_nc():
    if not _NC_CACHE:
        _NC_CACHE.append(_build_nc())
    return _NC_CACHE[0]


def _split12(v):
    """x = a + b with a = top-12-bit part (both fp32r-exact)."""
    a = np.floor(v * 4096.0) / np.float32(4096.0)
    a = a.astype(np.float32)
    b = (v - a).astype(np.float32)
    return a, b


def _split_f16(v):
    hi = np.float16(v).astype(np.float32)
    lo = (v - hi).astype(np.float32)
    return hi, lo


LAST_HW_NS = None


def kernel(xyz, xyz_prev, features, features_prev,
           W0, b0, g0, be0, W1, b1, g1, be1):
    global LAST_HW_NS
    xyz = np.asarray(xyz, np.float32)
    xyz_prev = np.asarray(xyz_prev, np.float32)
    features = np.asarray(features, np.float32)
    features_prev = np.asarray(features_prev, np.float32)
    W0 = np.asarray(W0, np.float32)
    W1 = np.asarray(W1, np.float32)

    w0pt = np.ascontiguousarray(W0[:, :CP].T).astype(np.float16)
    w0ft = np.ascontiguousarray(W0[:, CP:].T).astype(np.float16)
    w1t = np.ascontiguousarray(W1.T).astype(np.float16)
    eye3 = np.tile(np.eye(128, dtype=np.float32), (1, 3))
    eye3 = np.ascontiguousarray(
        np.concatenate([np.eye(128, dtype=np.float32)] * 3, axis=1))
    c0b = np.tile(np.array(C0S, np.uint32)[None, :], (128, 1))
    g0d = np.ascontiguousarray(np.asarray(g0, np.float32).reshape(2, 128).T)
    be0d = np.ascontiguousarray(np.asarray(be0, np.float32).reshape(2, 128).T)
    g1d = np.asarray(g1, np.float32).reshape(1, 128).T.copy()
    be1d = np.asarray(be1, np.float32).reshape(1, 128).T.copy()

    in_maps = []
    perms = []
    for bb_ in range(B):
        zq = xyz[bb_][:, 2]
        zp = xyz_prev[bb_][:, 2]
        pq = np.argsort(zq)
        ppm = np.argsort(zp)
        perms.append(pq)
        # center coords: halves the dynamic range of the norm/cross terms in
        # the split-fp32r distance matmul -> ~4x smaller absolute d2 error
        x = xyz[bb_][pq] - np.float32(0.5)   # [N, 3] sorted queries
        p = xyz_prev[bb_][ppm] - np.float32(0.5)  # [M, 3] sorted prev
        xa, xb = _split12(x)
        pa, pb = _split12(p)
        nx2 = (x * x).sum(-1, dtype=np.float32)
        np2 = (p * p).sum(-1, dtype=np.float32)
        nxh, nxl = _split_f16(nx2)
        nph, npl = _split_f16(np2)

        lhs16 = np.empty((16, N), np.float32)
        rhs16 = np.empty((16, M), np.float32)
        for c in range(3):
            lhs16[c] = 2.0 * xa[:, c]
            lhs16[3 + c] = 2.0 * xa[:, c]
            lhs16[6 + c] = 2.0 * xb[:, c]
            lhs16[9 + c] = 2.0 * xb[:, c]
            rhs16[c] = pa[:, c]
            rhs16[3 + c] = pb[:, c]
            rhs16[6 + c] = pa[:, c]
            rhs16[9 + c] = pb[:, c]
        lhs16[12] = 1.0
        lhs16[13] = 1.0
        lhs16[14] = -nxh
        lhs16[15] = -nxl
        rhs16[12] = -nph
        rhs16[13] = -npl
        rhs16[14] = 1.0
        rhs16[15] = 1.0

        feat_s = np.ascontiguousarray(features[bb_][:, pq]).astype(np.float16)
        fp_s = np.ascontiguousarray(features_prev[bb_][:, ppm]).astype(np.float16)

        in_maps.append({
            "lhs16": lhs16, "rhs16": rhs16,
            "feat": feat_s, "fp": fp_s,
            "w0pt": w0pt, "w0ft": w0ft, "w1t": w1t,
            "eye3": eye3, "c0b": c0b,
            "g0": g0d, "be0": be0d, "g1": g1d, "be1": be1d,
        })

    nc = _get_nc()
    trace = os.environ.get("KTRACE", "0") == "1"
    res = run_bass_kernel_spmd(nc, in_maps, list(range(B)), trace=trace)
    global LAST_RES, LAST_INMAPS, LAST_PERMS
    LAST_RES, LAST_INMAPS, LAST_PERMS = res, in_maps, perms
    if trace and res.exec_time_ns is not None:
        LAST_HW_NS = res.exec_time_ns
        if res.instructions_and_trace is not None:
            print("trace:", res.instructions_and_trace[1])
    out = np.empty((B, O1, N), np.float32)
    for bb_ in range(B):
        out[bb_][:, perms[bb_]] = res.results[bb_]["out"]
    return out
